# revision 13
# baseline (speedup 1.0000x reference)
"""Trainium2 Bass kernel for nn_DeformableGCN (GNN message passing).

Strategy (1D graph partitioning over 8 NeuronCores):
  - Destination nodes are assigned to cores/tiles via a degree-sorted
    permutation pi: each 128-node tile holds nodes with nearly equal
    in-degree (split by gather-table half), so each dst node's in-edges
    occupy its own SBUF partition across a minimal number of 128-edge
    chunks ("identity scatter": the segment-sum matmul uses a constant
    identity weight matrix - no per-chunk one-hot construction).
  - Per-edge source rows are fetched with the custom dma_gather
    instruction (int16 indices, 4 SWDGE queues). The node table is
    addressed in two halves (rows < 32768 / >= 32768) to fit int16;
    padding slots gather a guaranteed-zero fake-node row.
  - Each smoothing step computes the core's dst shard, then an
    AllGather rebuilds the full node table for the next step's gathers.
    The conv layers gather rows [h@W_lin | h@W_att_src] of per-node
    projected tables (AllGather'd); edge scores lrelu(u_src + a_dst)
    use the per-tile a_dst column, partition-aligned by construction.
"""
import os
import sys

sys.path.insert(0, "/opt/trn_rl_repo")

import numpy as np

import concourse.bass as bass
import concourse.bacc as bacc
import concourse.mybir as mybir
import concourse.tile as tile
from concourse.masks import make_identity

M = 8            # cores
P = 128          # partitions
LOB = 32768      # int16 table-half boundary (rows)
WIN = 32         # gather-call window, in 128-edge chunks
F32 = mybir.dt.float32
I16 = mybir.dt.int16
NEG_SLOPE = 0.01


# ------------------------------------------------------------- pjrt runner

class _Runner:
    """Builds the jitted PJRT callable once; repeated exec without retrace."""

    def __init__(self, nc, n_cores):
        import jax
        from jax.sharding import Mesh, PartitionSpec
        from jax.experimental.shard_map import shard_map
        from concourse.bass2jax import (
            install_neuronx_cc_hook, _bass_exec_p, partition_id_tensor)
        install_neuronx_cc_hook()
        self.jax = jax
        self.n_cores = n_cores
        in_names, out_names, out_avals, zero_outs = [], [], [], []
        partition_name = (nc.partition_id_tensor.name
                          if nc.partition_id_tensor else None)
        for alloc in nc.m.functions[0].allocations:
            if not isinstance(alloc, mybir.MemoryLocationSet):
                continue
            name = alloc.memorylocations[0].name
            if alloc.kind == "ExternalInput":
                if name != partition_name:
                    in_names.append(name)
            elif alloc.kind == "ExternalOutput":
                shape = tuple(alloc.tensor_shape)
                dtype = mybir.dt.np(alloc.dtype)
                out_names.append(name)
                out_avals.append(jax.core.ShapedArray(shape, dtype))
                zero_outs.append(np.zeros(shape, dtype))
        self.in_names, self.out_names = in_names, out_names
        self.zero_outs = zero_outs
        n_params = len(in_names)
        all_in_names = list(in_names) + list(out_names)
        if partition_name is not None:
            all_in_names.append(partition_name)

        def _body(*args):
            operands = list(args)
            if partition_name is not None:
                operands.append(partition_id_tensor())
            outs = _bass_exec_p.bind(
                *operands,
                out_avals=tuple(out_avals),
                in_names=tuple(all_in_names),
                out_names=tuple(out_names),
                lowering_input_output_aliases=(),
                sim_require_finite=True,
                sim_require_nnan=True,
                nc=nc,
            )
            return tuple(outs)

        donate = tuple(range(n_params, n_params + len(out_names)))
        devices = jax.devices()[:n_cores]
        self.mesh = Mesh(np.asarray(devices), ("core",))
        in_specs = (PartitionSpec("core"),) * (n_params + len(out_names))
        out_specs = (PartitionSpec("core"),) * len(out_names)
        self.fn = jax.jit(
            shard_map(_body, mesh=self.mesh, in_specs=in_specs,
                      out_specs=out_specs, check_rep=False),
            donate_argnums=donate, keep_unused=True)
        self._dev_inputs = None

    def place_inputs(self, in_maps):
        import jax
        from jax.sharding import PartitionSpec, NamedSharding
        per_core = [[np.asarray(m[n]) for n in self.in_names]
                    for m in in_maps]
        arrs = []
        for i, n in enumerate(self.in_names):
            concat = np.concatenate(
                [per_core[c][i] for c in range(self.n_cores)], axis=0)
            arrs.append(jax.device_put(
                concat, NamedSharding(self.mesh, PartitionSpec("core"))))
        for a in arrs:
            a.block_until_ready()
        self._dev_inputs = arrs

    def _zeros(self):
        return [np.zeros((self.n_cores * z.shape[0], *z.shape[1:]), z.dtype)
                for z in self.zero_outs]

    def exec_async(self):
        return self.fn(*self._dev_inputs, *self._zeros())

    def run(self, in_maps=None):
        if in_maps is not None:
            self.place_inputs(in_maps)
        outs = [np.asarray(o) for o in self.exec_async()]
        res = []
        for c in range(self.n_cores):
            d = {}
            for i, n in enumerate(self.out_names):
                per = outs[i].reshape(
                    (self.n_cores, outs[i].shape[0] // self.n_cores)
                    + outs[i].shape[1:])
                d[n] = per[c]
            res.append(d)
        return res

    def time_exec(self, k=8):
        import time
        o = self.exec_async()
        self.jax.block_until_ready(o)
        t0 = time.perf_counter()
        outs = [self.exec_async() for _ in range(k)]
        self.jax.block_until_ready(outs)
        return (time.perf_counter() - t0) / k


# ---------------------------------------------------------------- schedule

def _build_schedule(src, dst, n_nodes):
    """Host-side graph partitioning: permutation, slots, gather indices."""
    E = src.shape[0]
    NP = -(-n_nodes // (M * P)) * (M * P)
    if NP - n_nodes < 2:
        NP += M * P  # guarantee >=2 fake (zero) nodes for padding
    TPC = NP // (M * P)

    deg = np.bincount(dst, minlength=NP).astype(np.int64)

    # Pin the gather-table-half ("lo") membership by original id, so the
    # degree sort below can use the exact final per-half in-degrees.
    is_lo = np.zeros(NP, bool)
    if NP > LOB:
        if n_nodes >= LOB:
            is_lo[: LOB - 1] = True
            is_lo[n_nodes] = True       # one fake (zero) node in lo half
        else:
            is_lo[:LOB] = True          # includes some fakes
        n_lo_tiles = LOB // P
    else:
        is_lo[:] = True
        n_lo_tiles = NP // P
    assert is_lo.sum() == n_lo_tiles * P

    elo_n = is_lo[src]
    dlo = np.bincount(dst[elo_n], minlength=NP)
    dhi = deg - dlo

    # snake order: -dlo primary; within each dlo group alternate dhi
    # direction so dhi varies continuously across group boundaries
    # (minimizes per-tile max in-degree for tiles straddling groups)
    snake_dhi = np.where(dlo % 2 == 0, dhi, -dhi)
    keys = np.lexsort((-snake_dhi, -dlo))
    l_order = keys[is_lo[keys]]
    h_order = keys[~is_lo[keys]]

    # form tiles (consecutive 128 nodes of each pool), then jointly sort
    # all tiles and deal them to positions so that the 8 tiles at each
    # position have matched per-half max in-degrees (minimizes padding)
    l_tiles = l_order.reshape(-1, P)
    h_tiles = h_order.reshape(-1, P) if len(h_order) else \
        np.empty((0, P), np.int64)
    tiles = ([("L", t) for t in l_tiles] + [("H", t) for t in h_tiles])
    clo_t = np.array([dlo[t].max() for _, t in tiles])
    chi_t = np.array([dhi[t].max() for _, t in tiles])
    # banded 2D assignment: split clo-sorted tiles into bands of B
    # positions; within each band, distribute by chi with a min-increase
    # greedy (keeps both per-position maxes tight)
    quota_lo = [[c for c in range(M) if c * TPC + tau < n_lo_tiles]
                for tau in range(TPC)]
    quota_hi = [[c for c in range(M) if c * TPC + tau >= n_lo_tiles]
                for tau in range(TPC)]
    kinds = np.array([0 if k == "L" else 1 for k, _ in tiles])
    l_ids = np.flatnonzero(kinds == 0)
    h_ids = np.flatnonzero(kinds == 1)
    l_sorted = l_ids[np.lexsort((-chi_t[l_ids], -clo_t[l_ids]))]
    h_sorted = h_ids[np.lexsort((-chi_t[h_ids], -clo_t[h_ids]))]
    B = 8
    cur_lo = np.zeros(TPC, np.int64)
    cur_hi = np.zeros(TPC, np.int64)
    pi = np.empty(NP, np.int64)
    lp = hp = 0
    for b0 in range(0, TPC, B):
        taus = list(range(b0, min(b0 + B, TPC)))
        nL = sum(len(quota_lo[t]) for t in taus)
        nH = sum(len(quota_hi[t]) for t in taus)
        band = list(l_sorted[lp: lp + nL]) + list(h_sorted[hp: hp + nH])
        lp += nL
        hp += nH
        band.sort(key=lambda ti: -chi_t[ti])
        for ti in band:
            quota = quota_lo if kinds[ti] == 0 else quota_hi
            best, best_cost = -1, None
            for tau in taus:
                if not quota[tau]:
                    continue
                d = (max(cur_lo[tau], clo_t[ti]) - cur_lo[tau]
                     + max(cur_hi[tau], chi_t[ti]) - cur_hi[tau])
                used = 8 - len(quota_lo[tau]) - len(quota_hi[tau])
                cost = (d, -used)
                if best_cost is None or cost < best_cost:
                    best, best_cost = tau, cost
            tau = best
            c = quota[tau].pop()
            cur_lo[tau] = max(cur_lo[tau], clo_t[ti])
            cur_hi[tau] = max(cur_hi[tau], chi_t[ti])
            pi[tiles[ti][1]] = (c * TPC + tau) * P + np.arange(P)

    pi_src = pi[src]
    pi_dst = pi[dst]
    elo = pi_src < LOB
    assert np.array_equal(elo, elo_n)

    fakes_pi = pi[n_nodes:]
    lo_fakes = fakes_pi[fakes_pi < LOB]
    assert lo_fakes.size >= 1
    LO_PAD = int(lo_fakes[0])
    if NP > LOB:
        hi_fakes = fakes_pi[fakes_pi >= LOB]
        assert hi_fakes.size >= 1
        HI_PAD = int(hi_fakes[0]) - LOB
    else:
        HI_PAD = 0

    nlo = np.bincount(pi_dst[elo], minlength=NP)
    nhi = np.bincount(pi_dst[~elo], minlength=NP)
    CLO = nlo.reshape(M, TPC, P).max(axis=2).max(axis=0)
    CHI = nhi.reshape(M, TPC, P).max(axis=2).max(axis=0)
    base = np.zeros(TPC + 1, np.int64)
    base[1:] = np.cumsum(CLO + CHI)
    TOTC = int(base[-1])

    key = pi_dst * 2 + (~elo).astype(np.int64)
    eorder = np.argsort(key, kind="stable")
    ks = key[eorder]
    new_grp = np.ones(E, bool)
    new_grp[1:] = ks[1:] != ks[:-1]
    starts = np.flatnonzero(new_grp)
    grp_id = np.cumsum(new_grp) - 1
    rank_in_grp = np.arange(E) - starts[grp_id]
    tau_e = (pi_dst[eorder] % (TPC * P)) // P
    kchunk = rank_in_grp + np.where(ks % 2 == 0, 0, CLO[tau_e])
    core_e = pi_dst[eorder] // (TPC * P)
    j_e = pi_dst[eorder] % P
    slot = (base[tau_e] + kchunk) * P + j_e

    chunk_is_lo = np.zeros(TOTC, bool)
    for t in range(TPC):
        chunk_is_lo[base[t]: base[t] + CLO[t]] = True

    idx_flat = np.where(chunk_is_lo[None, :, None], np.int16(LO_PAD),
                        np.int16(HI_PAD)).astype(np.int16)
    idx_flat = np.broadcast_to(idx_flat, (M, TOTC, P)).reshape(M, TOTC * P)
    idx_flat = np.ascontiguousarray(idx_flat)
    vals = np.where(elo[eorder], pi_src[eorder], pi_src[eorder] - LOB)
    idx_flat[core_e, slot] = vals.astype(np.int16)

    lo_cids = np.flatnonzero(chunk_is_lo)
    hi_cids = np.flatnonzero(~chunk_is_lo)
    streams = {"lo": lo_cids, "hi": hi_cids}
    windows = []
    chunk_loc = {}
    col16 = 0
    for sname in ("lo", "hi"):
        cids = streams[sname]
        for wi0 in range(0, len(cids), WIN):
            wcids = cids[wi0: wi0 + WIN]
            swi = wi0 // WIN
            windows.append((sname, swi, len(wcids), col16))
            for sslot, cid in enumerate(wcids):
                chunk_loc[int(cid)] = (sname, swi, sslot)
            col16 += len(wcids) * P // 16
    TOT16 = col16

    idx_res = np.zeros((M, 128, TOT16), np.int16)
    for c in range(M):
        for (sname, swi, nch, off) in windows:
            cids = streams[sname][swi * WIN: swi * WIN + nch]
            block = idx_flat[c].reshape(TOTC, P)[cids].reshape(-1)
            wr = block.reshape(-1, 16).T
            idx_res[c, :, off: off + nch * P // 16] = np.tile(wr, (8, 1))

    rdeg_pi = np.empty(NP, np.float32)
    rdeg_pi[pi] = (1.0 / np.maximum(deg, 1.0)).astype(np.float32)
    rdeg_ct = rdeg_pi.reshape(M, TPC, P).transpose(0, 2, 1)

    return dict(
        E=E, NP=NP, TPC=TPC, TOTC=TOTC, TOT16=TOT16,
        pi=pi, CLO=CLO, CHI=CHI, base=base,
        windows=windows, chunk_loc=chunk_loc,
        streams=streams, idx_res=idx_res, rdeg_ct=np.ascontiguousarray(rdeg_ct),
    )


# ---------------------------------------------------------------- program

def _build_program(s, D, DH, DO, repeat=1):
    NP, TPC, TOT16 = s["NP"], s["TPC"], s["TOT16"]
    CLO, CHI, base = s["CLO"], s["CHI"], s["base"]
    windows, chunk_loc = s["windows"], s["chunk_loc"]
    NSH = TPC * P

    nc = bacc.Bacc("TRN2", target_bir_lowering=False, debug=False,
                   enable_asserts=False, num_devices=M, num_swdge_queues=4)

    x_full = nc.dram_tensor("x_full", [NP, D], F32, kind="ExternalInput")
    x_shard_t = nc.dram_tensor("x_shard_t", [P, TPC * D], F32,
                               kind="ExternalInput")
    idx_in = nc.dram_tensor("idx_in", [P, TOT16], I16, kind="ExternalInput")
    rdeg_in = nc.dram_tensor("rdeg_in", [P, TPC], F32, kind="ExternalInput")
    wcat1_in = nc.dram_tensor("wcat1_in", [D, D + 2], F32, kind="ExternalInput")
    wcat2_in = nc.dram_tensor("wcat2_in", [DH, DO + 2], F32,
                              kind="ExternalInput")
    params_in = nc.dram_tensor("params_in", [P, 2], F32, kind="ExternalInput")
    out_sh = nc.dram_tensor("out_sh", [NSH, DO], F32, kind="ExternalOutput")
    debug = os.environ.get("CC_GCN_DEBUG", "") == "1"
    if debug:
        dbg_h0 = nc.dram_tensor("dbg_h0", [NSH, D], F32, kind="ExternalOutput")
        dbg_acc = nc.dram_tensor("dbg_acc", [P, TPC * D], F32,
                                 kind="ExternalOutput")
        dbg_h1 = nc.dram_tensor("dbg_h1", [NSH, DH], F32, kind="ExternalOutput")

    RG = [list(range(M))]
    ROW1 = 2 * D  # conv1 table row width (f32): [hl(D) | u | pad]

    with tile.TileContext(nc) as tc:
        with (
            tc.tile_pool(name="consts", bufs=1) as cp,
            tc.tile_pool(name="glo", bufs=3) as glop,
            tc.tile_pool(name="ghi", bufs=3) as ghip,
            tc.tile_pool(name="work", bufs=3) as wp,
            tc.tile_pool(name="small", bufs=4) as sp,
            tc.tile_pool(name="fpsum", bufs=3, space="PSUM") as fpp,
            tc.tile_pool(name="tpsum", bufs=2, space="PSUM") as tpp,
            tc.tile_pool(name="mpsum", bufs=2, space="PSUM") as mpp,
            tc.tile_pool(name="dram", bufs=1, space="DRAM") as dp,
        ):
            ident = cp.tile([P, P], F32, name="ident")
            make_identity(nc, ident[:])
            idxt = cp.tile([P, TOT16], I16, name="idxt")
            nc.sync.dma_start(out=idxt[:], in_=idx_in[:])
            rdeg = cp.tile([P, TPC], F32, name="rdeg")
            nc.sync.dma_start(out=rdeg[:], in_=rdeg_in[:])
            wcat1 = cp.tile([D, D + 2], F32, name="wcat1")
            nc.sync.dma_start(out=wcat1[:], in_=wcat1_in[:])
            wcat2 = cp.tile([DH, DO + 2], F32, name="wcat2")
            nc.sync.dma_start(out=wcat2[:], in_=wcat2_in[:])
            params = cp.tile([P, 2], F32, name="params")
            nc.sync.dma_start(out=params[:], in_=params_in[:])
            acc = cp.tile([P, TPC * D], F32, name="acc")
            adst1 = cp.tile([P, TPC], F32, name="adst1")
            adst2 = cp.tile([P, TPC], F32, name="adst2")

            hin = dp.tile([NSH, D], F32, name="hin")
            t1in = dp.tile([NSH, ROW1], F32, name="t1in")
            t2in = dp.tile([NSH, DH], F32, name="t2in")

            def emit_gathers(table_ap, drow, tag):
                bufs = {}
                qn = 0
                for (sname, swi, nch, off) in windows:
                    pool = glop if sname == "lo" else ghip
                    b = pool.tile([P, WIN * ROW1], F32,
                                  name=f"g{tag}{sname}{swi}", tag=f"g{sname}")
                    num = nch * P
                    if sname == "lo":
                        src_ap = table_ap[0:min(LOB, NP), :]
                    else:
                        src_ap = table_ap[LOB:NP, :]
                    nc.gpsimd.dma_gather(
                        out_ap=b[:, : nch * drow].rearrange(
                            "p (c d) -> p c d", d=drow),
                        in_ap=src_ap,
                        idxs_ap=idxt[:, off: off + nch * P // 16],
                        num_idxs=num,
                        num_idxs_reg=num,
                        elem_size=drow,
                        single_packet=False,
                        queue_num=qn % 4,
                    )
                    qn += 1
                    bufs[(sname, swi)] = b
                return bufs

            def chunk_groups(t):
                runs = []
                for cid in range(int(base[t]), int(base[t + 1])):
                    sname, swi, sslot = chunk_loc[cid]
                    if runs and runs[-1][0] == (sname, swi) and \
                            runs[-1][1] + runs[-1][2] == sslot:
                        runs[-1] = (runs[-1][0], runs[-1][1], runs[-1][2] + 1)
                    else:
                        runs.append(((sname, swi), sslot, 1))
                return runs

            def proj_tile(t, xt_ap, wcat_t, din, dout, rowbuf_w, dest,
                          adst_sb, bcol, tag):
                """rows [X@W | u]; saves a_dst column (+bias)."""
                tp = tpp.tile([din, P], F32, name=f"tp{tag}_{t}", tag="tps")
                nc.tensor.transpose(out=tp[:], in_=xt_ap, identity=ident[:])
                xT = sp.tile([din, P], F32, name=f"xT{tag}_{t}", tag="xT")
                nc.scalar.activation(out=xT[:], in_=tp[:],
                                     func=mybir.ActivationFunctionType.Copy)
                mp = mpp.tile([P, dout + 2], F32, name=f"mp{tag}_{t}",
                              tag="mps")
                nc.tensor.matmul(out=mp[:], lhsT=xT[:], rhs=wcat_t[:],
                                 start=True, stop=True)
                row = wp.tile([P, rowbuf_w], F32, name=f"row{tag}_{t}",
                              tag=f"row{tag}")
                nc.scalar.activation(out=row[:, : dout + 1],
                                     in_=mp[:, : dout + 1],
                                     func=mybir.ActivationFunctionType.Copy)
                nc.vector.tensor_scalar(
                    out=adst_sb[:, t:t + 1], in0=mp[:, dout + 1: dout + 2],
                    scalar1=bcol, scalar2=None, op0=mybir.AluOpType.add)
                nc.sync.dma_start(out=dest[t * P:(t + 1) * P, :], in_=row[:])

            def smoothing_pass(table_ap, pnum, rep, need_ag=True):
                bufs = emit_gathers(table_ap, D, f"s{pnum}r{rep}")
                for t in range(TPC):
                    nch = int(CLO[t] + CHI[t])
                    h = sp.tile([P, D], F32, name=f"h{pnum}_{t}_{rep}",
                                tag="h")
                    if nch == 0:
                        nc.vector.memset(h[:], 0.0)
                    else:
                        ps = fpp.tile([P, D], F32, name=f"ps{pnum}_{t}_{rep}",
                                      tag="fps")
                        k = 0
                        for (bk, s0, n) in chunk_groups(t):
                            b = bufs[bk]
                            for si in range(s0, s0 + n):
                                nc.tensor.matmul(
                                    out=ps[:], lhsT=ident[:],
                                    rhs=b[:, si * D:(si + 1) * D],
                                    start=(k == 0), stop=(k == nch - 1))
                                k += 1
                        nc.vector.tensor_scalar(
                            out=h[:], in0=ps[:], scalar1=rdeg[:, t:t + 1],
                            scalar2=None, op0=mybir.AluOpType.mult)
                        nc.vector.tensor_tensor(
                            out=acc[:, t * D:(t + 1) * D],
                            in0=acc[:, t * D:(t + 1) * D], in1=h[:],
                            op=mybir.AluOpType.add)
                    if need_ag:
                        nc.sync.dma_start(out=hin[t * P:(t + 1) * P, :],
                                          in_=h[:])
                    if debug and pnum == 0:
                        nc.sync.dma_start(out=dbg_h0[t * P:(t + 1) * P, :],
                                          in_=h[:])

            def conv_pass(table_ap, drow, df, adst_sb, pnum, post_fn, rep):
                bufs = emit_gathers(table_ap, drow, f"c{pnum}r{rep}")
                for t in range(TPC):
                    nch = int(CLO[t] + CHI[t])
                    if nch == 0:
                        post_fn(t, None)
                        continue
                    ps = fpp.tile([P, df], F32, name=f"cp{pnum}_{t}_{rep}",
                                  tag="fps")
                    k = 0
                    for (bk, s0, n) in chunk_groups(t):
                        b = bufs[bk]
                        g3 = b[:, s0 * drow:(s0 + n) * drow].rearrange(
                            "p (c d) -> p c d", d=drow)
                        z = sp.tile([P, WIN], F32,
                                    name=f"z{pnum}_{t}_{k}_{rep}", tag="z")
                        nc.vector.tensor_scalar(
                            out=z[:, :n].rearrange("p (c u) -> p c u", u=1),
                            in0=g3[:, :, df:df + 1],
                            scalar1=adst_sb[:, t:t + 1], scalar2=None,
                            op0=mybir.AluOpType.add)
                        sc = sp.tile([P, WIN], F32,
                                     name=f"sc{pnum}_{t}_{k}_{rep}", tag="sc")
                        nc.scalar.activation(
                            out=sc[:, :n], in_=z[:, :n],
                            func=mybir.ActivationFunctionType.Lrelu,
                            alpha=NEG_SLOPE)
                        w8 = wp.tile([P, WIN * D], F32,
                                     name=f"w8{pnum}_{t}_{k}_{rep}", tag="w8")
                        nc.vector.tensor_tensor(
                            out=w8[:, : n * df].rearrange(
                                "p (c d) -> p c d", d=df),
                            in0=g3[:, :, 0:df],
                            in1=sc[:, :n].to_broadcast([P, n, df]),
                            op=mybir.AluOpType.mult)
                        for si in range(n):
                            nc.tensor.matmul(
                                out=ps[:], lhsT=ident[:],
                                rhs=w8[:, si * df:(si + 1) * df],
                                start=(k == 0), stop=(k == nch - 1))
                            k += 1
                    post_fn(t, ps)

            for rep in range(repeat):
                htab1 = dp.tile([NP, D], F32, name=f"htab1_{rep}",
                                addr_space="Shared")
                htab2 = dp.tile([NP, D], F32, name=f"htab2_{rep}",
                                addr_space="Shared")
                t1tab = dp.tile([NP, ROW1], F32, name=f"t1tab_{rep}",
                                addr_space="Shared")
                t2tab = dp.tile([NP, DH], F32, name=f"t2tab_{rep}",
                                addr_space="Shared")
                nc.sync.dma_start(out=acc[:], in_=x_shard_t[:])

                smoothing_pass(x_full.ap(), 0, rep)
                nc.gpsimd.collective_compute(
                    "AllGather", mybir.AluOpType.bypass,
                    ins=[hin.opt()], outs=[htab1.opt()], replica_groups=RG)
                smoothing_pass(htab1[:], 1, rep)
                nc.gpsimd.collective_compute(
                    "AllGather", mybir.AluOpType.bypass,
                    ins=[hin.opt()], outs=[htab2.opt()], replica_groups=RG)
                smoothing_pass(htab2[:], 2, rep, need_ag=False)

                if debug:
                    nc.sync.dma_start(out=dbg_acc[:], in_=acc[:])
                for t in range(TPC):
                    proj_tile(t, acc[:, t * D:(t + 1) * D], wcat1, D, D,
                              ROW1, t1in, adst1, params[:, 0:1],
                              f"t1_{rep}")
                nc.gpsimd.collective_compute(
                    "AllGather", mybir.AluOpType.bypass,
                    ins=[t1in.opt()], outs=[t1tab.opt()], replica_groups=RG)

                def post1(t, ps, rep=rep):
                    h1 = sp.tile([P, DH], F32, name=f"h1_{t}_{rep}", tag="h1")
                    if ps is None:
                        nc.vector.memset(h1[:], 0.0)
                    else:
                        nc.scalar.activation(
                            out=h1[:], in_=ps[:],
                            func=mybir.ActivationFunctionType.Relu)
                    if debug:
                        nc.sync.dma_start(out=dbg_h1[t * P:(t + 1) * P, :],
                                          in_=h1[:])
                    proj_tile(t, h1[:], wcat2, DH, DO, DH, t2in, adst2,
                              params[:, 1:2], f"t2_{rep}")

                conv_pass(t1tab[:], ROW1, D, adst1, 1, post1, rep)
                nc.gpsimd.collective_compute(
                    "AllGather", mybir.AluOpType.bypass,
                    ins=[t2in.opt()], outs=[t2tab.opt()], replica_groups=RG)

                def post2(t, ps, rep=rep):
                    o = sp.tile([P, DO], F32, name=f"o_{t}_{rep}", tag="o")
                    if ps is None:
                        nc.vector.memset(o[:], 0.0)
                    else:
                        nc.scalar.activation(
                            out=o[:], in_=ps[:],
                            func=mybir.ActivationFunctionType.Copy)
                    nc.sync.dma_start(out=out_sh[t * P:(t + 1) * P, :],
                                      in_=o[:])

                conv_pass(t2tab[:], DH, DO, adst2, 2, post2, rep)

    nc.compile()
    return nc


# ---------------------------------------------------------------- driver

_CACHE = {}


def _get_runner(s, D, DH, DO, repeat):
    key = (s["NP"], s["TOTC"], s["TOT16"], tuple(int(v) for v in s["CLO"]),
           tuple(int(v) for v in s["CHI"]), D, DH, DO, repeat)
    if key not in _CACHE:
        nc = _build_program(s, D, DH, DO, repeat)
        _CACHE[key] = _Runner(nc, M)
    return _CACHE[key]


def _prep_inputs(s, x, W_att1, b_att1, W_lin1, W_att2, b_att2, W_lin2):
    NP, TPC = s["NP"], s["TPC"]
    N, D = x.shape
    DH = W_lin1.shape[1]
    DO = W_lin2.shape[1]
    pi = s["pi"]

    x_full = np.zeros((NP, D), np.float32)
    x_full[pi[:N]] = x
    x_sh = x_full.reshape(M, TPC, P, D)

    wcat1 = np.concatenate(
        [W_lin1, W_att1[:D, :1], W_att1[D:, :1]], axis=1) * 0.25
    wcat2 = np.concatenate(
        [W_lin2, W_att2[:DH, :1], W_att2[DH:, :1]], axis=1)
    params = np.zeros((P, 2), np.float32)
    params[:, 0] = float(np.asarray(b_att1).reshape(-1)[0])
    params[:, 1] = float(np.asarray(b_att2).reshape(-1)[0])

    in_maps = []
    for c in range(M):
        in_maps.append({
            "x_full": x_full,
            "x_shard_t": np.ascontiguousarray(
                x_sh[c].transpose(1, 0, 2)).reshape(P, TPC * D),
            "idx_in": s["idx_res"][c],
            "rdeg_in": s["rdeg_ct"][c],
            "wcat1_in": wcat1.astype(np.float32),
            "wcat2_in": wcat2.astype(np.float32),
            "params_in": params,
        })
    return in_maps


def kernel(x, edge_index, W_att1, b_att1, W_lin1, W_att2, b_att2, W_lin2):
    x = np.asarray(x, np.float32)
    edge_index = np.asarray(edge_index)
    N, D = x.shape
    W_lin1 = np.asarray(W_lin1, np.float32)
    W_lin2 = np.asarray(W_lin2, np.float32)
    DH = W_lin1.shape[1]
    DO = W_lin2.shape[1]
    src = edge_index[0].astype(np.int64)
    dst = edge_index[1].astype(np.int64)

    s = _build_schedule(src, dst, N)
    repeat = int(os.environ.get("CC_GCN_REPEAT", "1"))
    r = _get_runner(s, D, DH, DO, repeat)
    in_maps = _prep_inputs(s, x, np.asarray(W_att1, np.float32),
                           np.asarray(b_att1, np.float32), W_lin1,
                           np.asarray(W_att2, np.float32),
                           np.asarray(b_att2, np.float32), W_lin2)
    res = r.run(in_maps)

    pi = s["pi"]
    out_pi = np.concatenate([res[c]["out_sh"] for c in range(M)], axis=0)
    return np.ascontiguousarray(out_pi[pi[:N]]).astype(np.float32)


# revision 14
# speedup vs baseline: 1.5975x; 1.5975x over previous
"""Trainium2 Bass kernel for nn_DeformableGCN (GNN message passing).

Strategy (1D graph partitioning over 8 NeuronCores):
  - Destination nodes are assigned to cores/tiles via a degree-sorted
    permutation pi: each 128-node tile holds nodes with nearly equal
    in-degree (split by gather-table half), so each dst node's in-edges
    occupy its own SBUF partition across a minimal number of 128-edge
    chunks ("identity scatter": the segment-sum matmul uses a constant
    identity weight matrix - no per-chunk one-hot construction).
  - Per-edge source rows are fetched with the custom dma_gather
    instruction (int16 indices, 4 SWDGE queues). The node table is
    addressed in two halves (rows < 32768 / >= 32768) to fit int16;
    padding slots gather a guaranteed-zero fake-node row.
  - Each smoothing step computes the core's dst shard, then an
    AllGather rebuilds the full node table for the next step's gathers.
    The conv layers gather rows [h@W_lin | h@W_att_src] of per-node
    projected tables (AllGather'd); edge scores lrelu(u_src + a_dst)
    use the per-tile a_dst column, partition-aligned by construction.
"""
import os
import sys

sys.path.insert(0, "/opt/trn_rl_repo")

import numpy as np

import concourse.bass as bass
import concourse.bacc as bacc
import concourse.mybir as mybir
import concourse.tile as tile
from concourse.masks import make_identity

M = 8            # cores
P = 128          # partitions
LOB = 32768      # int16 table-half boundary (rows)
WIN = 32         # gather-call window, in 128-edge chunks
F32 = mybir.dt.float32
I16 = mybir.dt.int16
NEG_SLOPE = 0.01


# ------------------------------------------------------------- pjrt runner

class _Runner:
    """Builds the jitted PJRT callable once; repeated exec without retrace."""

    def __init__(self, nc, n_cores):
        import jax
        from jax.sharding import Mesh, PartitionSpec
        from jax.experimental.shard_map import shard_map
        from concourse.bass2jax import (
            install_neuronx_cc_hook, _bass_exec_p, partition_id_tensor)
        install_neuronx_cc_hook()
        self.jax = jax
        self.n_cores = n_cores
        in_names, out_names, out_avals, zero_outs = [], [], [], []
        partition_name = (nc.partition_id_tensor.name
                          if nc.partition_id_tensor else None)
        for alloc in nc.m.functions[0].allocations:
            if not isinstance(alloc, mybir.MemoryLocationSet):
                continue
            name = alloc.memorylocations[0].name
            if alloc.kind == "ExternalInput":
                if name != partition_name:
                    in_names.append(name)
            elif alloc.kind == "ExternalOutput":
                shape = tuple(alloc.tensor_shape)
                dtype = mybir.dt.np(alloc.dtype)
                out_names.append(name)
                out_avals.append(jax.core.ShapedArray(shape, dtype))
                zero_outs.append(np.zeros(shape, dtype))
        self.in_names, self.out_names = in_names, out_names
        self.zero_outs = zero_outs
        n_params = len(in_names)
        all_in_names = list(in_names) + list(out_names)
        if partition_name is not None:
            all_in_names.append(partition_name)

        def _body(*args):
            operands = list(args)
            if partition_name is not None:
                operands.append(partition_id_tensor())
            outs = _bass_exec_p.bind(
                *operands,
                out_avals=tuple(out_avals),
                in_names=tuple(all_in_names),
                out_names=tuple(out_names),
                lowering_input_output_aliases=(),
                sim_require_finite=True,
                sim_require_nnan=True,
                nc=nc,
            )
            return tuple(outs)

        donate = tuple(range(n_params, n_params + len(out_names)))
        devices = jax.devices()[:n_cores]
        self.mesh = Mesh(np.asarray(devices), ("core",))
        in_specs = (PartitionSpec("core"),) * (n_params + len(out_names))
        out_specs = (PartitionSpec("core"),) * len(out_names)
        self.fn = jax.jit(
            shard_map(_body, mesh=self.mesh, in_specs=in_specs,
                      out_specs=out_specs, check_rep=False),
            donate_argnums=donate, keep_unused=True)
        self._dev_inputs = None

    def place_inputs(self, in_maps):
        import jax
        from jax.sharding import PartitionSpec, NamedSharding
        per_core = [[np.asarray(m[n]) for n in self.in_names]
                    for m in in_maps]
        arrs = []
        for i, n in enumerate(self.in_names):
            concat = np.concatenate(
                [per_core[c][i] for c in range(self.n_cores)], axis=0)
            arrs.append(jax.device_put(
                concat, NamedSharding(self.mesh, PartitionSpec("core"))))
        for a in arrs:
            a.block_until_ready()
        self._dev_inputs = arrs

    def _zeros(self):
        return [np.zeros((self.n_cores * z.shape[0], *z.shape[1:]), z.dtype)
                for z in self.zero_outs]

    def exec_async(self):
        return self.fn(*self._dev_inputs, *self._zeros())

    def run(self, in_maps=None):
        if in_maps is not None:
            self.place_inputs(in_maps)
        outs = [np.asarray(o) for o in self.exec_async()]
        res = []
        for c in range(self.n_cores):
            d = {}
            for i, n in enumerate(self.out_names):
                per = outs[i].reshape(
                    (self.n_cores, outs[i].shape[0] // self.n_cores)
                    + outs[i].shape[1:])
                d[n] = per[c]
            res.append(d)
        return res

    def time_exec(self, k=8):
        import time
        o = self.exec_async()
        self.jax.block_until_ready(o)
        t0 = time.perf_counter()
        outs = [self.exec_async() for _ in range(k)]
        self.jax.block_until_ready(outs)
        return (time.perf_counter() - t0) / k


# ---------------------------------------------------------------- schedule

def _build_schedule(src, dst, n_nodes):
    """Host-side graph partitioning: permutation, slots, gather indices."""
    E = src.shape[0]
    NP = -(-n_nodes // (M * P)) * (M * P)
    if NP - n_nodes < 2:
        NP += M * P  # guarantee >=2 fake (zero) nodes for padding
    TPC = NP // (M * P)

    deg = np.bincount(dst, minlength=NP).astype(np.int64)

    # Pin the gather-table-half ("lo") membership by original id, so the
    # degree sort below can use the exact final per-half in-degrees.
    is_lo = np.zeros(NP, bool)
    if NP > LOB:
        if n_nodes >= LOB:
            is_lo[: LOB - 1] = True
            is_lo[n_nodes] = True       # one fake (zero) node in lo half
        else:
            is_lo[:LOB] = True          # includes some fakes
        n_lo_tiles = LOB // P
    else:
        is_lo[:] = True
        n_lo_tiles = NP // P
    assert is_lo.sum() == n_lo_tiles * P

    elo_n = is_lo[src]
    dlo = np.bincount(dst[elo_n], minlength=NP)
    dhi = deg - dlo

    # snake order: -dlo primary; within each dlo group alternate dhi
    # direction so dhi varies continuously across group boundaries
    # (minimizes per-tile max in-degree for tiles straddling groups)
    snake_dhi = np.where(dlo % 2 == 0, dhi, -dhi)
    keys = np.lexsort((-snake_dhi, -dlo))
    l_order = keys[is_lo[keys]]
    h_order = keys[~is_lo[keys]]

    # form tiles (consecutive 128 nodes of each pool), then jointly sort
    # all tiles and deal them to positions so that the 8 tiles at each
    # position have matched per-half max in-degrees (minimizes padding)
    l_tiles = l_order.reshape(-1, P)
    h_tiles = h_order.reshape(-1, P) if len(h_order) else \
        np.empty((0, P), np.int64)
    tiles = ([("L", t) for t in l_tiles] + [("H", t) for t in h_tiles])
    clo_t = np.array([dlo[t].max() for _, t in tiles])
    chi_t = np.array([dhi[t].max() for _, t in tiles])
    # banded 2D assignment: split clo-sorted tiles into bands of B
    # positions; within each band, distribute by chi with a min-increase
    # greedy (keeps both per-position maxes tight)
    quota_lo = [[c for c in range(M) if c * TPC + tau < n_lo_tiles]
                for tau in range(TPC)]
    quota_hi = [[c for c in range(M) if c * TPC + tau >= n_lo_tiles]
                for tau in range(TPC)]
    kinds = np.array([0 if k == "L" else 1 for k, _ in tiles])
    l_ids = np.flatnonzero(kinds == 0)
    h_ids = np.flatnonzero(kinds == 1)
    l_sorted = l_ids[np.lexsort((-chi_t[l_ids], -clo_t[l_ids]))]
    h_sorted = h_ids[np.lexsort((-chi_t[h_ids], -clo_t[h_ids]))]
    B = 8
    cur_lo = np.zeros(TPC, np.int64)
    cur_hi = np.zeros(TPC, np.int64)
    assign = np.zeros(len(tiles), np.int64)
    lp = hp = 0
    for b0 in range(0, TPC, B):
        taus = list(range(b0, min(b0 + B, TPC)))
        rem_lo = {t: len(quota_lo[t]) for t in taus}
        rem_hi = {t: len(quota_hi[t]) for t in taus}
        nL = sum(rem_lo.values())
        nH = sum(rem_hi.values())
        band = list(l_sorted[lp: lp + nL]) + list(h_sorted[hp: hp + nH])
        lp += nL
        hp += nH
        band.sort(key=lambda ti: -chi_t[ti])
        for ti in band:
            rem = rem_lo if kinds[ti] == 0 else rem_hi
            best, best_cost = -1, None
            for tau in taus:
                if rem[tau] == 0:
                    continue
                d = (max(cur_lo[tau], clo_t[ti]) - cur_lo[tau]
                     + max(cur_hi[tau], chi_t[ti]) - cur_hi[tau])
                used = 16 - rem_lo[tau] - rem_hi[tau]
                cost = (d, -used)
                if best_cost is None or cost < best_cost:
                    best, best_cost = tau, cost
            tau = best
            rem[tau] -= 1
            cur_lo[tau] = max(cur_lo[tau], clo_t[ti])
            cur_hi[tau] = max(cur_hi[tau], chi_t[ti])
            assign[ti] = tau

    # local-search refinement: swap same-kind tiles between positions
    members = [[[] for _ in range(TPC)] for _ in range(2)]
    for ti in range(len(tiles)):
        members[kinds[ti]][assign[ti]].append(ti)

    def pos_cost(t):
        tis = members[0][t] + members[1][t]
        return (max((clo_t[i] for i in tis), default=0)
                + max((chi_t[i] for i in tis), default=0))

    rng_ls = np.random.default_rng(0)
    for _ in range(60000):
        k = int(rng_ls.integers(0, 2))
        p, q = (int(v) for v in rng_ls.integers(0, TPC, 2))
        if p == q or not members[k][p] or not members[k][q]:
            continue
        i = members[k][p][int(rng_ls.integers(len(members[k][p])))]
        j = members[k][q][int(rng_ls.integers(len(members[k][q])))]
        before = pos_cost(p) + pos_cost(q)
        members[k][p].remove(i)
        members[k][q].remove(j)
        members[k][p].append(j)
        members[k][q].append(i)
        if pos_cost(p) + pos_cost(q) >= before:
            members[k][p].remove(j)
            members[k][q].remove(i)
            members[k][p].append(i)
            members[k][q].append(j)

    pi = np.empty(NP, np.int64)
    for k, quota in ((0, quota_lo), (1, quota_hi)):
        for tau in range(TPC):
            for ti in members[k][tau]:
                c = quota[tau].pop()
                pi[tiles[ti][1]] = (c * TPC + tau) * P + np.arange(P)
    assert all(not q for q in quota_lo) and all(not q for q in quota_hi)

    pi_src = pi[src]
    pi_dst = pi[dst]
    elo = pi_src < LOB
    assert np.array_equal(elo, elo_n)

    fakes_pi = pi[n_nodes:]
    lo_fakes = fakes_pi[fakes_pi < LOB]
    assert lo_fakes.size >= 1
    LO_PAD = int(lo_fakes[0])
    if NP > LOB:
        hi_fakes = fakes_pi[fakes_pi >= LOB]
        assert hi_fakes.size >= 1
        HI_PAD = int(hi_fakes[0]) - LOB
    else:
        HI_PAD = 0

    nlo = np.bincount(pi_dst[elo], minlength=NP)
    nhi = np.bincount(pi_dst[~elo], minlength=NP)
    CLO = nlo.reshape(M, TPC, P).max(axis=2).max(axis=0)
    CHI = nhi.reshape(M, TPC, P).max(axis=2).max(axis=0)
    base = np.zeros(TPC + 1, np.int64)
    base[1:] = np.cumsum(CLO + CHI)
    TOTC = int(base[-1])

    key = pi_dst * 2 + (~elo).astype(np.int64)
    eorder = np.argsort(key, kind="stable")
    ks = key[eorder]
    new_grp = np.ones(E, bool)
    new_grp[1:] = ks[1:] != ks[:-1]
    starts = np.flatnonzero(new_grp)
    grp_id = np.cumsum(new_grp) - 1
    rank_in_grp = np.arange(E) - starts[grp_id]
    tau_e = (pi_dst[eorder] % (TPC * P)) // P
    kchunk = rank_in_grp + np.where(ks % 2 == 0, 0, CLO[tau_e])
    core_e = pi_dst[eorder] // (TPC * P)
    j_e = pi_dst[eorder] % P
    slot = (base[tau_e] + kchunk) * P + j_e

    chunk_is_lo = np.zeros(TOTC, bool)
    for t in range(TPC):
        chunk_is_lo[base[t]: base[t] + CLO[t]] = True

    idx_flat = np.where(chunk_is_lo[None, :, None], np.int16(LO_PAD),
                        np.int16(HI_PAD)).astype(np.int16)
    idx_flat = np.broadcast_to(idx_flat, (M, TOTC, P)).reshape(M, TOTC * P)
    idx_flat = np.ascontiguousarray(idx_flat)
    vals = np.where(elo[eorder], pi_src[eorder], pi_src[eorder] - LOB)
    idx_flat[core_e, slot] = vals.astype(np.int16)

    lo_cids = np.flatnonzero(chunk_is_lo)
    hi_cids = np.flatnonzero(~chunk_is_lo)
    streams = {"lo": lo_cids, "hi": hi_cids}
    windows = []
    chunk_loc = {}
    col16 = 0
    for sname in ("lo", "hi"):
        cids = streams[sname]
        for wi0 in range(0, len(cids), WIN):
            wcids = cids[wi0: wi0 + WIN]
            swi = wi0 // WIN
            windows.append((sname, swi, len(wcids), col16))
            for sslot, cid in enumerate(wcids):
                chunk_loc[int(cid)] = (sname, swi, sslot)
            col16 += len(wcids) * P // 16
    TOT16 = col16

    idx_res = np.zeros((M, 128, TOT16), np.int16)
    for c in range(M):
        for (sname, swi, nch, off) in windows:
            cids = streams[sname][swi * WIN: swi * WIN + nch]
            block = idx_flat[c].reshape(TOTC, P)[cids].reshape(-1)
            wr = block.reshape(-1, 16).T
            idx_res[c, :, off: off + nch * P // 16] = np.tile(wr, (8, 1))

    rdeg_pi = np.empty(NP, np.float32)
    rdeg_pi[pi] = (1.0 / np.maximum(deg, 1.0)).astype(np.float32)
    rdeg_ct = rdeg_pi.reshape(M, TPC, P).transpose(0, 2, 1)

    return dict(
        E=E, NP=NP, TPC=TPC, TOTC=TOTC, TOT16=TOT16,
        pi=pi, CLO=CLO, CHI=CHI, base=base,
        windows=windows, chunk_loc=chunk_loc,
        streams=streams, idx_res=idx_res, rdeg_ct=np.ascontiguousarray(rdeg_ct),
    )


# ---------------------------------------------------------------- program

def _build_program(s, D, DH, DO, repeat=1):
    NP, TPC, TOT16 = s["NP"], s["TPC"], s["TOT16"]
    CLO, CHI, base = s["CLO"], s["CHI"], s["base"]
    windows, chunk_loc = s["windows"], s["chunk_loc"]
    NSH = TPC * P

    nc = bacc.Bacc("TRN2", target_bir_lowering=False, debug=False,
                   enable_asserts=False, num_devices=M, num_swdge_queues=4)

    x_full = nc.dram_tensor("x_full", [NP, D], F32, kind="ExternalInput")
    x_shard_t = nc.dram_tensor("x_shard_t", [P, TPC * D], F32,
                               kind="ExternalInput")
    idx_in = nc.dram_tensor("idx_in", [P, TOT16], I16, kind="ExternalInput")
    rdeg_in = nc.dram_tensor("rdeg_in", [P, TPC], F32, kind="ExternalInput")
    wcat1_in = nc.dram_tensor("wcat1_in", [D, D + 2], F32, kind="ExternalInput")
    wcat2_in = nc.dram_tensor("wcat2_in", [DH, DO + 2], F32,
                              kind="ExternalInput")
    params_in = nc.dram_tensor("params_in", [P, 2], F32, kind="ExternalInput")
    out_sh = nc.dram_tensor("out_sh", [NSH, DO], F32, kind="ExternalOutput")
    debug = os.environ.get("CC_GCN_DEBUG", "") == "1"
    if debug:
        dbg_h0 = nc.dram_tensor("dbg_h0", [NSH, D], F32, kind="ExternalOutput")
        dbg_acc = nc.dram_tensor("dbg_acc", [P, TPC * D], F32,
                                 kind="ExternalOutput")
        dbg_h1 = nc.dram_tensor("dbg_h1", [NSH, DH], F32, kind="ExternalOutput")

    RG = [list(range(M))]
    ROW1 = 2 * D  # conv1 table row width (f32): [hl(D) | u | pad]

    with tile.TileContext(nc) as tc:
        with (
            tc.tile_pool(name="consts", bufs=1) as cp,
            tc.tile_pool(name="glo", bufs=3) as glop,
            tc.tile_pool(name="ghi", bufs=3) as ghip,
            tc.tile_pool(name="work", bufs=3) as wp,
            tc.tile_pool(name="small", bufs=4) as sp,
            tc.tile_pool(name="fpsum", bufs=3, space="PSUM") as fpp,
            tc.tile_pool(name="tpsum", bufs=2, space="PSUM") as tpp,
            tc.tile_pool(name="mpsum", bufs=2, space="PSUM") as mpp,
            tc.tile_pool(name="dram", bufs=1, space="DRAM") as dp,
        ):
            ident = cp.tile([P, P], F32, name="ident")
            make_identity(nc, ident[:])
            idxt = cp.tile([P, TOT16], I16, name="idxt")
            nc.sync.dma_start(out=idxt[:], in_=idx_in[:])
            rdeg = cp.tile([P, TPC], F32, name="rdeg")
            nc.sync.dma_start(out=rdeg[:], in_=rdeg_in[:])
            wcat1 = cp.tile([D, D + 2], F32, name="wcat1")
            nc.sync.dma_start(out=wcat1[:], in_=wcat1_in[:])
            wcat2 = cp.tile([DH, DO + 2], F32, name="wcat2")
            nc.sync.dma_start(out=wcat2[:], in_=wcat2_in[:])
            params = cp.tile([P, 2], F32, name="params")
            nc.sync.dma_start(out=params[:], in_=params_in[:])
            acc = cp.tile([P, TPC * D], F32, name="acc")
            adst1 = cp.tile([P, TPC], F32, name="adst1")
            adst2 = cp.tile([P, TPC], F32, name="adst2")

            hin = dp.tile([NSH, D], F32, name="hin")
            t1in = dp.tile([NSH, ROW1], F32, name="t1in")
            t2in = dp.tile([NSH, DH], F32, name="t2in")

            def emit_gathers(table_ap, drow, tag):
                bufs = {}
                qn = 0
                for (sname, swi, nch, off) in windows:
                    pool = glop if sname == "lo" else ghip
                    b = pool.tile([P, WIN * ROW1], F32,
                                  name=f"g{tag}{sname}{swi}", tag=f"g{sname}")
                    num = nch * P
                    if sname == "lo":
                        src_ap = table_ap[0:min(LOB, NP), :]
                    else:
                        src_ap = table_ap[LOB:NP, :]
                    nc.gpsimd.dma_gather(
                        out_ap=b[:, : nch * drow].rearrange(
                            "p (c d) -> p c d", d=drow),
                        in_ap=src_ap,
                        idxs_ap=idxt[:, off: off + nch * P // 16],
                        num_idxs=num,
                        num_idxs_reg=num,
                        elem_size=drow,
                        single_packet=False,
                        queue_num=qn % 4,
                    )
                    qn += 1
                    bufs[(sname, swi)] = b
                return bufs

            def chunk_groups(t):
                runs = []
                for cid in range(int(base[t]), int(base[t + 1])):
                    sname, swi, sslot = chunk_loc[cid]
                    if runs and runs[-1][0] == (sname, swi) and \
                            runs[-1][1] + runs[-1][2] == sslot:
                        runs[-1] = (runs[-1][0], runs[-1][1], runs[-1][2] + 1)
                    else:
                        runs.append(((sname, swi), sslot, 1))
                return runs

            def proj_tile(t, xt_ap, wcat_t, din, dout, rowbuf_w, dest,
                          adst_sb, bcol, tag):
                """rows [X@W | u]; saves a_dst column (+bias)."""
                tp = tpp.tile([din, P], F32, name=f"tp{tag}_{t}", tag="tps")
                nc.tensor.transpose(out=tp[:], in_=xt_ap, identity=ident[:])
                xT = sp.tile([din, P], F32, name=f"xT{tag}_{t}", tag="xT")
                nc.scalar.activation(out=xT[:], in_=tp[:],
                                     func=mybir.ActivationFunctionType.Copy)
                mp = mpp.tile([P, dout + 2], F32, name=f"mp{tag}_{t}",
                              tag="mps")
                nc.tensor.matmul(out=mp[:], lhsT=xT[:], rhs=wcat_t[:],
                                 start=True, stop=True)
                row = wp.tile([P, rowbuf_w], F32, name=f"row{tag}_{t}",
                              tag=f"row{tag}")
                nc.scalar.activation(out=row[:, : dout + 1],
                                     in_=mp[:, : dout + 1],
                                     func=mybir.ActivationFunctionType.Copy)
                nc.vector.tensor_scalar(
                    out=adst_sb[:, t:t + 1], in0=mp[:, dout + 1: dout + 2],
                    scalar1=bcol, scalar2=None, op0=mybir.AluOpType.add)
                nc.sync.dma_start(out=dest[t * P:(t + 1) * P, :], in_=row[:])

            def smoothing_pass(table_ap, pnum, rep, need_ag=True):
                bufs = emit_gathers(table_ap, D, f"s{pnum}r{rep}")
                for t in range(TPC):
                    nch = int(CLO[t] + CHI[t])
                    h = sp.tile([P, D], F32, name=f"h{pnum}_{t}_{rep}",
                                tag="h")
                    if nch == 0:
                        nc.vector.memset(h[:], 0.0)
                    else:
                        ps = fpp.tile([P, D], F32, name=f"ps{pnum}_{t}_{rep}",
                                      tag="fps")
                        k = 0
                        for (bk, s0, n) in chunk_groups(t):
                            b = bufs[bk]
                            for si in range(s0, s0 + n):
                                nc.tensor.matmul(
                                    out=ps[:], lhsT=ident[:],
                                    rhs=b[:, si * D:(si + 1) * D],
                                    start=(k == 0), stop=(k == nch - 1))
                                k += 1
                        nc.vector.tensor_scalar(
                            out=h[:], in0=ps[:], scalar1=rdeg[:, t:t + 1],
                            scalar2=None, op0=mybir.AluOpType.mult)
                        nc.vector.tensor_tensor(
                            out=acc[:, t * D:(t + 1) * D],
                            in0=acc[:, t * D:(t + 1) * D], in1=h[:],
                            op=mybir.AluOpType.add)
                    if need_ag:
                        nc.sync.dma_start(out=hin[t * P:(t + 1) * P, :],
                                          in_=h[:])
                    if debug and pnum == 0:
                        nc.sync.dma_start(out=dbg_h0[t * P:(t + 1) * P, :],
                                          in_=h[:])

            def conv_pass(table_ap, drow, df, adst_sb, pnum, post_fn, rep):
                bufs = emit_gathers(table_ap, drow, f"c{pnum}r{rep}")
                for t in range(TPC):
                    nch = int(CLO[t] + CHI[t])
                    if nch == 0:
                        post_fn(t, None)
                        continue
                    ps = fpp.tile([P, df], F32, name=f"cp{pnum}_{t}_{rep}",
                                  tag="fps")
                    k = 0
                    for (bk, s0, n) in chunk_groups(t):
                        b = bufs[bk]
                        g3 = b[:, s0 * drow:(s0 + n) * drow].rearrange(
                            "p (c d) -> p c d", d=drow)
                        z = sp.tile([P, WIN], F32,
                                    name=f"z{pnum}_{t}_{k}_{rep}", tag="z")
                        nc.vector.tensor_scalar(
                            out=z[:, :n].rearrange("p (c u) -> p c u", u=1),
                            in0=g3[:, :, df:df + 1],
                            scalar1=adst_sb[:, t:t + 1], scalar2=None,
                            op0=mybir.AluOpType.add)
                        sc = sp.tile([P, WIN], F32,
                                     name=f"sc{pnum}_{t}_{k}_{rep}", tag="sc")
                        nc.scalar.activation(
                            out=sc[:, :n], in_=z[:, :n],
                            func=mybir.ActivationFunctionType.Lrelu,
                            alpha=NEG_SLOPE)
                        w8 = wp.tile([P, WIN * D], F32,
                                     name=f"w8{pnum}_{t}_{k}_{rep}", tag="w8")
                        nc.vector.tensor_tensor(
                            out=w8[:, : n * df].rearrange(
                                "p (c d) -> p c d", d=df),
                            in0=g3[:, :, 0:df],
                            in1=sc[:, :n].to_broadcast([P, n, df]),
                            op=mybir.AluOpType.mult)
                        for si in range(n):
                            nc.tensor.matmul(
                                out=ps[:], lhsT=ident[:],
                                rhs=w8[:, si * df:(si + 1) * df],
                                start=(k == 0), stop=(k == nch - 1))
                            k += 1
                    post_fn(t, ps)

            for rep in range(repeat):
                htab1 = dp.tile([NP, D], F32, name=f"htab1_{rep}",
                                addr_space="Shared")
                htab2 = dp.tile([NP, D], F32, name=f"htab2_{rep}",
                                addr_space="Shared")
                t1tab = dp.tile([NP, ROW1], F32, name=f"t1tab_{rep}",
                                addr_space="Shared")
                t2tab = dp.tile([NP, DH], F32, name=f"t2tab_{rep}",
                                addr_space="Shared")
                nc.sync.dma_start(out=acc[:], in_=x_shard_t[:])

                smoothing_pass(x_full.ap(), 0, rep)
                nc.gpsimd.collective_compute(
                    "AllGather", mybir.AluOpType.bypass,
                    ins=[hin.opt()], outs=[htab1.opt()], replica_groups=RG)
                smoothing_pass(htab1[:], 1, rep)
                nc.gpsimd.collective_compute(
                    "AllGather", mybir.AluOpType.bypass,
                    ins=[hin.opt()], outs=[htab2.opt()], replica_groups=RG)
                smoothing_pass(htab2[:], 2, rep, need_ag=False)

                if debug:
                    nc.sync.dma_start(out=dbg_acc[:], in_=acc[:])
                for t in range(TPC):
                    proj_tile(t, acc[:, t * D:(t + 1) * D], wcat1, D, D,
                              ROW1, t1in, adst1, params[:, 0:1],
                              f"t1_{rep}")
                nc.gpsimd.collective_compute(
                    "AllGather", mybir.AluOpType.bypass,
                    ins=[t1in.opt()], outs=[t1tab.opt()], replica_groups=RG)

                def post1(t, ps, rep=rep):
                    h1 = sp.tile([P, DH], F32, name=f"h1_{t}_{rep}", tag="h1")
                    if ps is None:
                        nc.vector.memset(h1[:], 0.0)
                    else:
                        nc.scalar.activation(
                            out=h1[:], in_=ps[:],
                            func=mybir.ActivationFunctionType.Relu)
                    if debug:
                        nc.sync.dma_start(out=dbg_h1[t * P:(t + 1) * P, :],
                                          in_=h1[:])
                    proj_tile(t, h1[:], wcat2, DH, DO, DH, t2in, adst2,
                              params[:, 1:2], f"t2_{rep}")

                conv_pass(t1tab[:], ROW1, D, adst1, 1, post1, rep)
                nc.gpsimd.collective_compute(
                    "AllGather", mybir.AluOpType.bypass,
                    ins=[t2in.opt()], outs=[t2tab.opt()], replica_groups=RG)

                def post2(t, ps, rep=rep):
                    o = sp.tile([P, DO], F32, name=f"o_{t}_{rep}", tag="o")
                    if ps is None:
                        nc.vector.memset(o[:], 0.0)
                    else:
                        nc.scalar.activation(
                            out=o[:], in_=ps[:],
                            func=mybir.ActivationFunctionType.Copy)
                    nc.sync.dma_start(out=out_sh[t * P:(t + 1) * P, :],
                                      in_=o[:])

                conv_pass(t2tab[:], DH, DO, adst2, 2, post2, rep)

    nc.compile()
    return nc


# ---------------------------------------------------------------- driver

_CACHE = {}


def _get_runner(s, D, DH, DO, repeat):
    key = (s["NP"], s["TOTC"], s["TOT16"], tuple(int(v) for v in s["CLO"]),
           tuple(int(v) for v in s["CHI"]), D, DH, DO, repeat)
    if key not in _CACHE:
        nc = _build_program(s, D, DH, DO, repeat)
        _CACHE[key] = _Runner(nc, M)
    return _CACHE[key]


def _prep_inputs(s, x, W_att1, b_att1, W_lin1, W_att2, b_att2, W_lin2):
    NP, TPC = s["NP"], s["TPC"]
    N, D = x.shape
    DH = W_lin1.shape[1]
    DO = W_lin2.shape[1]
    pi = s["pi"]

    x_full = np.zeros((NP, D), np.float32)
    x_full[pi[:N]] = x
    x_sh = x_full.reshape(M, TPC, P, D)

    wcat1 = np.concatenate(
        [W_lin1, W_att1[:D, :1], W_att1[D:, :1]], axis=1) * 0.25
    wcat2 = np.concatenate(
        [W_lin2, W_att2[:DH, :1], W_att2[DH:, :1]], axis=1)
    params = np.zeros((P, 2), np.float32)
    params[:, 0] = float(np.asarray(b_att1).reshape(-1)[0])
    params[:, 1] = float(np.asarray(b_att2).reshape(-1)[0])

    in_maps = []
    for c in range(M):
        in_maps.append({
            "x_full": x_full,
            "x_shard_t": np.ascontiguousarray(
                x_sh[c].transpose(1, 0, 2)).reshape(P, TPC * D),
            "idx_in": s["idx_res"][c],
            "rdeg_in": s["rdeg_ct"][c],
            "wcat1_in": wcat1.astype(np.float32),
            "wcat2_in": wcat2.astype(np.float32),
            "params_in": params,
        })
    return in_maps


def kernel(x, edge_index, W_att1, b_att1, W_lin1, W_att2, b_att2, W_lin2):
    x = np.asarray(x, np.float32)
    edge_index = np.asarray(edge_index)
    N, D = x.shape
    W_lin1 = np.asarray(W_lin1, np.float32)
    W_lin2 = np.asarray(W_lin2, np.float32)
    DH = W_lin1.shape[1]
    DO = W_lin2.shape[1]
    src = edge_index[0].astype(np.int64)
    dst = edge_index[1].astype(np.int64)

    s = _build_schedule(src, dst, N)
    repeat = int(os.environ.get("CC_GCN_REPEAT", "1"))
    r = _get_runner(s, D, DH, DO, repeat)
    in_maps = _prep_inputs(s, x, np.asarray(W_att1, np.float32),
                           np.asarray(b_att1, np.float32), W_lin1,
                           np.asarray(W_att2, np.float32),
                           np.asarray(b_att2, np.float32), W_lin2)
    res = r.run(in_maps)

    pi = s["pi"]
    out_pi = np.concatenate([res[c]["out_sh"] for c in range(M)], axis=0)
    return np.ascontiguousarray(out_pi[pi[:N]]).astype(np.float32)


# revision 16
# speedup vs baseline: 1.6447x; 1.0295x over previous
"""Trainium2 Bass kernel for nn_DeformableGCN (GNN message passing).

Strategy (1D graph partitioning over 8 NeuronCores):
  - Destination nodes are assigned to cores/tiles via a degree-sorted
    permutation pi: each 128-node tile holds nodes with nearly equal
    in-degree (split by gather-table half), so each dst node's in-edges
    occupy its own SBUF partition across a minimal number of 128-edge
    chunks ("identity scatter": the segment-sum matmul uses a constant
    identity weight matrix - no per-chunk one-hot construction).
  - Per-edge source rows are fetched with the custom dma_gather
    instruction (int16 indices, 4 SWDGE queues). The node table is
    addressed in two halves (rows < 32768 / >= 32768) to fit int16;
    padding slots gather a guaranteed-zero fake-node row.
  - Each smoothing step computes the core's dst shard, then an
    AllGather rebuilds the full node table for the next step's gathers.
    The conv layers gather rows [h@W_lin | h@W_att_src] of per-node
    projected tables (AllGather'd); edge scores lrelu(u_src + a_dst)
    use the per-tile a_dst column, partition-aligned by construction.
"""
import os
import sys

sys.path.insert(0, "/opt/trn_rl_repo")

import numpy as np

import concourse.bass as bass
import concourse.bacc as bacc
import concourse.mybir as mybir
import concourse.tile as tile
from concourse.masks import make_identity

M = 8            # cores
P = 128          # partitions
LOB = 32768      # int16 table-half boundary (rows)
WIN = 32         # gather-call window, in 128-edge chunks
F32 = mybir.dt.float32
I16 = mybir.dt.int16
NEG_SLOPE = 0.01


# ------------------------------------------------------------- pjrt runner

class _Runner:
    """Builds the jitted PJRT callable once; repeated exec without retrace."""

    def __init__(self, nc, n_cores):
        import jax
        from jax.sharding import Mesh, PartitionSpec
        from jax.experimental.shard_map import shard_map
        from concourse.bass2jax import (
            install_neuronx_cc_hook, _bass_exec_p, partition_id_tensor)
        install_neuronx_cc_hook()
        self.jax = jax
        self.n_cores = n_cores
        in_names, out_names, out_avals, zero_outs = [], [], [], []
        partition_name = (nc.partition_id_tensor.name
                          if nc.partition_id_tensor else None)
        for alloc in nc.m.functions[0].allocations:
            if not isinstance(alloc, mybir.MemoryLocationSet):
                continue
            name = alloc.memorylocations[0].name
            if alloc.kind == "ExternalInput":
                if name != partition_name:
                    in_names.append(name)
            elif alloc.kind == "ExternalOutput":
                shape = tuple(alloc.tensor_shape)
                dtype = mybir.dt.np(alloc.dtype)
                out_names.append(name)
                out_avals.append(jax.core.ShapedArray(shape, dtype))
                zero_outs.append(np.zeros(shape, dtype))
        self.in_names, self.out_names = in_names, out_names
        self.zero_outs = zero_outs
        n_params = len(in_names)
        all_in_names = list(in_names) + list(out_names)
        if partition_name is not None:
            all_in_names.append(partition_name)

        def _body(*args):
            operands = list(args)
            if partition_name is not None:
                operands.append(partition_id_tensor())
            outs = _bass_exec_p.bind(
                *operands,
                out_avals=tuple(out_avals),
                in_names=tuple(all_in_names),
                out_names=tuple(out_names),
                lowering_input_output_aliases=(),
                sim_require_finite=True,
                sim_require_nnan=True,
                nc=nc,
            )
            return tuple(outs)

        donate = tuple(range(n_params, n_params + len(out_names)))
        devices = jax.devices()[:n_cores]
        self.mesh = Mesh(np.asarray(devices), ("core",))
        in_specs = (PartitionSpec("core"),) * (n_params + len(out_names))
        out_specs = (PartitionSpec("core"),) * len(out_names)
        self.fn = jax.jit(
            shard_map(_body, mesh=self.mesh, in_specs=in_specs,
                      out_specs=out_specs, check_rep=False),
            donate_argnums=donate, keep_unused=True)
        self._dev_inputs = None

    def place_inputs(self, in_maps):
        import jax
        from jax.sharding import PartitionSpec, NamedSharding
        per_core = [[np.asarray(m[n]) for n in self.in_names]
                    for m in in_maps]
        arrs = []
        for i, n in enumerate(self.in_names):
            concat = np.concatenate(
                [per_core[c][i] for c in range(self.n_cores)], axis=0)
            arrs.append(jax.device_put(
                concat, NamedSharding(self.mesh, PartitionSpec("core"))))
        for a in arrs:
            a.block_until_ready()
        self._dev_inputs = arrs

    def _zeros(self):
        return [np.zeros((self.n_cores * z.shape[0], *z.shape[1:]), z.dtype)
                for z in self.zero_outs]

    def exec_async(self):
        return self.fn(*self._dev_inputs, *self._zeros())

    def run(self, in_maps=None):
        if in_maps is not None:
            self.place_inputs(in_maps)
        outs = [np.asarray(o) for o in self.exec_async()]
        res = []
        for c in range(self.n_cores):
            d = {}
            for i, n in enumerate(self.out_names):
                per = outs[i].reshape(
                    (self.n_cores, outs[i].shape[0] // self.n_cores)
                    + outs[i].shape[1:])
                d[n] = per[c]
            res.append(d)
        return res

    def time_exec(self, k=8):
        import time
        o = self.exec_async()
        self.jax.block_until_ready(o)
        t0 = time.perf_counter()
        outs = [self.exec_async() for _ in range(k)]
        self.jax.block_until_ready(outs)
        return (time.perf_counter() - t0) / k


# ---------------------------------------------------------------- schedule

def _build_schedule(src, dst, n_nodes):
    """Host-side graph partitioning: permutation, slots, gather indices."""
    E = src.shape[0]
    NP = -(-n_nodes // (M * P)) * (M * P)
    if NP - n_nodes < 2:
        NP += M * P  # guarantee >=2 fake (zero) nodes for padding
    TPC = NP // (M * P)

    deg = np.bincount(dst, minlength=NP).astype(np.int64)

    # Pin the gather-table-half ("lo") membership by original id, so the
    # degree sort below can use the exact final per-half in-degrees.
    is_lo = np.zeros(NP, bool)
    if NP > LOB:
        if n_nodes >= LOB:
            is_lo[: LOB - 1] = True
            is_lo[n_nodes] = True       # one fake (zero) node in lo half
        else:
            is_lo[:LOB] = True          # includes some fakes
        n_lo_tiles = LOB // P
    else:
        is_lo[:] = True
        n_lo_tiles = NP // P
    assert is_lo.sum() == n_lo_tiles * P

    elo_n = is_lo[src]
    dlo = np.bincount(dst[elo_n], minlength=NP)
    dhi = deg - dlo

    # snake order: -dlo primary; within each dlo group alternate dhi
    # direction so dhi varies continuously across group boundaries
    # (minimizes per-tile max in-degree for tiles straddling groups)
    snake_dhi = np.where(dlo % 2 == 0, dhi, -dhi)
    keys = np.lexsort((-snake_dhi, -dlo))
    l_order = keys[is_lo[keys]]
    h_order = keys[~is_lo[keys]]

    # form tiles (consecutive 128 nodes of each pool), then jointly sort
    # all tiles and deal them to positions so that the 8 tiles at each
    # position have matched per-half max in-degrees (minimizes padding)
    l_tiles = l_order.reshape(-1, P)
    h_tiles = h_order.reshape(-1, P) if len(h_order) else \
        np.empty((0, P), np.int64)
    tiles = ([("L", t) for t in l_tiles] + [("H", t) for t in h_tiles])
    clo_t = np.array([dlo[t].max() for _, t in tiles])
    chi_t = np.array([dhi[t].max() for _, t in tiles])
    # banded 2D assignment: split clo-sorted tiles into bands of B
    # positions; within each band, distribute by chi with a min-increase
    # greedy (keeps both per-position maxes tight)
    quota_lo = [[c for c in range(M) if c * TPC + tau < n_lo_tiles]
                for tau in range(TPC)]
    quota_hi = [[c for c in range(M) if c * TPC + tau >= n_lo_tiles]
                for tau in range(TPC)]
    kinds = np.array([0 if k == "L" else 1 for k, _ in tiles])
    l_ids = np.flatnonzero(kinds == 0)
    h_ids = np.flatnonzero(kinds == 1)
    l_sorted = l_ids[np.lexsort((-chi_t[l_ids], -clo_t[l_ids]))]
    h_sorted = h_ids[np.lexsort((-chi_t[h_ids], -clo_t[h_ids]))]
    B = 8
    cur_lo = np.zeros(TPC, np.int64)
    cur_hi = np.zeros(TPC, np.int64)
    assign = np.zeros(len(tiles), np.int64)
    lp = hp = 0
    for b0 in range(0, TPC, B):
        taus = list(range(b0, min(b0 + B, TPC)))
        rem_lo = {t: len(quota_lo[t]) for t in taus}
        rem_hi = {t: len(quota_hi[t]) for t in taus}
        nL = sum(rem_lo.values())
        nH = sum(rem_hi.values())
        band = list(l_sorted[lp: lp + nL]) + list(h_sorted[hp: hp + nH])
        lp += nL
        hp += nH
        band.sort(key=lambda ti: -chi_t[ti])
        for ti in band:
            rem = rem_lo if kinds[ti] == 0 else rem_hi
            best, best_cost = -1, None
            for tau in taus:
                if rem[tau] == 0:
                    continue
                d = (max(cur_lo[tau], clo_t[ti]) - cur_lo[tau]
                     + max(cur_hi[tau], chi_t[ti]) - cur_hi[tau])
                used = 16 - rem_lo[tau] - rem_hi[tau]
                cost = (d, -used)
                if best_cost is None or cost < best_cost:
                    best, best_cost = tau, cost
            tau = best
            rem[tau] -= 1
            cur_lo[tau] = max(cur_lo[tau], clo_t[ti])
            cur_hi[tau] = max(cur_hi[tau], chi_t[ti])
            assign[ti] = tau

    # local-search refinement: swap same-kind tiles between positions
    members = [[[] for _ in range(TPC)] for _ in range(2)]
    for ti in range(len(tiles)):
        members[kinds[ti]][assign[ti]].append(ti)

    def pos_cost(t):
        tis = members[0][t] + members[1][t]
        return (max((clo_t[i] for i in tis), default=0)
                + max((chi_t[i] for i in tis), default=0))

    rng_ls = np.random.default_rng(0)
    for _ in range(60000):
        k = int(rng_ls.integers(0, 2))
        p, q = (int(v) for v in rng_ls.integers(0, TPC, 2))
        if p == q or not members[k][p] or not members[k][q]:
            continue
        i = members[k][p][int(rng_ls.integers(len(members[k][p])))]
        j = members[k][q][int(rng_ls.integers(len(members[k][q])))]
        before = pos_cost(p) + pos_cost(q)
        members[k][p].remove(i)
        members[k][q].remove(j)
        members[k][p].append(j)
        members[k][q].append(i)
        if pos_cost(p) + pos_cost(q) >= before:
            members[k][p].remove(j)
            members[k][q].remove(i)
            members[k][p].append(i)
            members[k][q].append(j)

    pi = np.empty(NP, np.int64)
    for k, quota in ((0, quota_lo), (1, quota_hi)):
        for tau in range(TPC):
            for ti in members[k][tau]:
                c = quota[tau].pop()
                pi[tiles[ti][1]] = (c * TPC + tau) * P + np.arange(P)
    assert all(not q for q in quota_lo) and all(not q for q in quota_hi)

    pi_src = pi[src]
    pi_dst = pi[dst]
    elo = pi_src < LOB
    assert np.array_equal(elo, elo_n)

    fakes_pi = pi[n_nodes:]
    lo_fakes = fakes_pi[fakes_pi < LOB]
    assert lo_fakes.size >= 1
    LO_PAD = int(lo_fakes[0])
    if NP > LOB:
        hi_fakes = fakes_pi[fakes_pi >= LOB]
        assert hi_fakes.size >= 1
        HI_PAD = int(hi_fakes[0]) - LOB
    else:
        HI_PAD = 0

    nlo = np.bincount(pi_dst[elo], minlength=NP)
    nhi = np.bincount(pi_dst[~elo], minlength=NP)
    CLO = nlo.reshape(M, TPC, P).max(axis=2).max(axis=0)
    CHI = nhi.reshape(M, TPC, P).max(axis=2).max(axis=0)
    base = np.zeros(TPC + 1, np.int64)
    base[1:] = np.cumsum(CLO + CHI)
    TOTC = int(base[-1])

    key = pi_dst * 2 + (~elo).astype(np.int64)
    eorder = np.argsort(key, kind="stable")
    ks = key[eorder]
    new_grp = np.ones(E, bool)
    new_grp[1:] = ks[1:] != ks[:-1]
    starts = np.flatnonzero(new_grp)
    grp_id = np.cumsum(new_grp) - 1
    rank_in_grp = np.arange(E) - starts[grp_id]
    tau_e = (pi_dst[eorder] % (TPC * P)) // P
    kchunk = rank_in_grp + np.where(ks % 2 == 0, 0, CLO[tau_e])
    core_e = pi_dst[eorder] // (TPC * P)
    j_e = pi_dst[eorder] % P
    slot = (base[tau_e] + kchunk) * P + j_e

    chunk_is_lo = np.zeros(TOTC, bool)
    for t in range(TPC):
        chunk_is_lo[base[t]: base[t] + CLO[t]] = True

    idx_flat = np.where(chunk_is_lo[None, :, None], np.int16(LO_PAD),
                        np.int16(HI_PAD)).astype(np.int16)
    idx_flat = np.broadcast_to(idx_flat, (M, TOTC, P)).reshape(M, TOTC * P)
    idx_flat = np.ascontiguousarray(idx_flat)
    vals = np.where(elo[eorder], pi_src[eorder], pi_src[eorder] - LOB)
    idx_flat[core_e, slot] = vals.astype(np.int16)

    lo_cids = np.flatnonzero(chunk_is_lo)
    hi_cids = np.flatnonzero(~chunk_is_lo)
    streams = {"lo": lo_cids, "hi": hi_cids}
    windows = []
    chunk_loc = {}
    col16 = 0
    for sname in ("lo", "hi"):
        cids = streams[sname]
        for wi0 in range(0, len(cids), WIN):
            wcids = cids[wi0: wi0 + WIN]
            swi = wi0 // WIN
            windows.append((sname, swi, len(wcids), col16))
            for sslot, cid in enumerate(wcids):
                chunk_loc[int(cid)] = (sname, swi, sslot)
            col16 += len(wcids) * P // 16
    TOT16 = col16

    idx_res = np.zeros((M, 128, TOT16), np.int16)
    for c in range(M):
        for (sname, swi, nch, off) in windows:
            cids = streams[sname][swi * WIN: swi * WIN + nch]
            block = idx_flat[c].reshape(TOTC, P)[cids].reshape(-1)
            wr = block.reshape(-1, 16).T
            idx_res[c, :, off: off + nch * P // 16] = np.tile(wr, (8, 1))

    rdeg_pi = np.empty(NP, np.float32)
    rdeg_pi[pi] = (1.0 / np.maximum(deg, 1.0)).astype(np.float32)
    rdeg_ct = rdeg_pi.reshape(M, TPC, P).transpose(0, 2, 1)

    return dict(
        E=E, NP=NP, TPC=TPC, TOTC=TOTC, TOT16=TOT16,
        pi=pi, CLO=CLO, CHI=CHI, base=base,
        windows=windows, chunk_loc=chunk_loc,
        streams=streams, idx_res=idx_res, rdeg_ct=np.ascontiguousarray(rdeg_ct),
    )


# ---------------------------------------------------------------- program

def _build_program(s, D, DH, DO, repeat=1):
    NP, TPC, TOT16 = s["NP"], s["TPC"], s["TOT16"]
    CLO, CHI, base = s["CLO"], s["CHI"], s["base"]
    windows, chunk_loc = s["windows"], s["chunk_loc"]
    NSH = TPC * P

    nc = bacc.Bacc("TRN2", target_bir_lowering=False, debug=False,
                   enable_asserts=False, num_devices=M, num_swdge_queues=4)

    x_full = nc.dram_tensor("x_full", [NP, D], F32, kind="ExternalInput")
    x_shard_t = nc.dram_tensor("x_shard_t", [P, TPC * D], F32,
                               kind="ExternalInput")
    idx_in = nc.dram_tensor("idx_in", [P, TOT16], I16, kind="ExternalInput")
    rdeg_in = nc.dram_tensor("rdeg_in", [P, TPC], F32, kind="ExternalInput")
    wcat1_in = nc.dram_tensor("wcat1_in", [D, D + 2], F32, kind="ExternalInput")
    wcat2_in = nc.dram_tensor("wcat2_in", [DH, DO + 2], F32,
                              kind="ExternalInput")
    params_in = nc.dram_tensor("params_in", [P, 2], F32, kind="ExternalInput")
    out_sh = nc.dram_tensor("out_sh", [NSH, DO], F32, kind="ExternalOutput")
    debug = os.environ.get("CC_GCN_DEBUG", "") == "1"
    if debug:
        dbg_h0 = nc.dram_tensor("dbg_h0", [NSH, D], F32, kind="ExternalOutput")
        dbg_acc = nc.dram_tensor("dbg_acc", [P, TPC * D], F32,
                                 kind="ExternalOutput")
        dbg_h1 = nc.dram_tensor("dbg_h1", [NSH, DH], F32, kind="ExternalOutput")

    RG = [list(range(M))]
    ROW1 = 2 * D  # conv1 table row width (f32): [hl(D) | u | pad]

    with tile.TileContext(nc) as tc:
        with (
            tc.tile_pool(name="consts", bufs=1) as cp,
            tc.tile_pool(name="glo", bufs=3) as glop,
            tc.tile_pool(name="ghi", bufs=3) as ghip,
            tc.tile_pool(name="work", bufs=3) as wp,
            tc.tile_pool(name="small", bufs=4) as sp,
            tc.tile_pool(name="fpsum", bufs=3, space="PSUM") as fpp,
            tc.tile_pool(name="tpsum", bufs=2, space="PSUM") as tpp,
            tc.tile_pool(name="mpsum", bufs=2, space="PSUM") as mpp,
            tc.tile_pool(name="dram", bufs=1, space="DRAM") as dp,
        ):
            ident = cp.tile([P, P], F32, name="ident")
            make_identity(nc, ident[:])
            idxt = cp.tile([P, TOT16], I16, name="idxt")
            nc.sync.dma_start(out=idxt[:], in_=idx_in[:])
            rdeg = cp.tile([P, TPC], F32, name="rdeg")
            nc.sync.dma_start(out=rdeg[:], in_=rdeg_in[:])
            wcat1 = cp.tile([D, D + 2], F32, name="wcat1")
            nc.sync.dma_start(out=wcat1[:], in_=wcat1_in[:])
            wcat2 = cp.tile([DH, DO + 2], F32, name="wcat2")
            nc.sync.dma_start(out=wcat2[:], in_=wcat2_in[:])
            params = cp.tile([P, 2], F32, name="params")
            nc.sync.dma_start(out=params[:], in_=params_in[:])
            acc = cp.tile([P, TPC * D], F32, name="acc")
            adst1 = cp.tile([P, TPC], F32, name="adst1")
            adst2 = cp.tile([P, TPC], F32, name="adst2")

            hin = dp.tile([NSH, D], F32, name="hin")
            t1in = dp.tile([NSH, ROW1], F32, name="t1in")
            t2in = dp.tile([NSH, DH], F32, name="t2in")

            def emit_gathers(table_ap, drow, tag):
                bufs = {}
                qn = 0
                for (sname, swi, nch, off) in windows:
                    pool = glop if sname == "lo" else ghip
                    b = pool.tile([P, WIN * ROW1], F32,
                                  name=f"g{tag}{sname}{swi}", tag=f"g{sname}")
                    num = nch * P
                    if sname == "lo":
                        src_ap = table_ap[0:min(LOB, NP), :]
                    else:
                        src_ap = table_ap[LOB:NP, :]
                    nc.gpsimd.dma_gather(
                        out_ap=b[:, : nch * drow].rearrange(
                            "p (c d) -> p c d", d=drow),
                        in_ap=src_ap,
                        idxs_ap=idxt[:, off: off + nch * P // 16],
                        num_idxs=num,
                        num_idxs_reg=num,
                        elem_size=drow,
                        single_packet=False,
                        queue_num=qn % 4,
                    )
                    qn += 1
                    bufs[(sname, swi)] = b
                return bufs

            def chunk_groups(t):
                runs = []
                for cid in range(int(base[t]), int(base[t + 1])):
                    sname, swi, sslot = chunk_loc[cid]
                    if runs and runs[-1][0] == (sname, swi) and \
                            runs[-1][1] + runs[-1][2] == sslot:
                        runs[-1] = (runs[-1][0], runs[-1][1], runs[-1][2] + 1)
                    else:
                        runs.append(((sname, swi), sslot, 1))
                return runs

            def proj_tile(t, xt_ap, wcat_t, din, dout, rowbuf_w, dest,
                          adst_sb, bcol, tag):
                """rows [X@W | u]; saves a_dst column (+bias)."""
                tp = tpp.tile([din, P], F32, name=f"tp{tag}_{t}", tag="tps")
                nc.tensor.transpose(out=tp[:], in_=xt_ap, identity=ident[:])
                xT = sp.tile([din, P], F32, name=f"xT{tag}_{t}", tag="xT")
                nc.scalar.activation(out=xT[:], in_=tp[:],
                                     func=mybir.ActivationFunctionType.Copy)
                mp = mpp.tile([P, dout + 2], F32, name=f"mp{tag}_{t}",
                              tag="mps")
                nc.tensor.matmul(out=mp[:], lhsT=xT[:], rhs=wcat_t[:],
                                 start=True, stop=True)
                row = wp.tile([P, rowbuf_w], F32, name=f"row{tag}_{t}",
                              tag=f"row{tag}")
                nc.scalar.activation(out=row[:, : dout + 1],
                                     in_=mp[:, : dout + 1],
                                     func=mybir.ActivationFunctionType.Copy)
                nc.vector.tensor_scalar(
                    out=adst_sb[:, t:t + 1], in0=mp[:, dout + 1: dout + 2],
                    scalar1=bcol, scalar2=None, op0=mybir.AluOpType.add)
                nc.sync.dma_start(out=dest[t * P:(t + 1) * P, :], in_=row[:])

            def smoothing_pass(table_ap, pnum, rep, need_ag=True):
                bufs = emit_gathers(table_ap, D, f"s{pnum}r{rep}")
                for t in range(TPC):
                    nch = int(CLO[t] + CHI[t])
                    h = sp.tile([P, D], F32, name=f"h{pnum}_{t}_{rep}",
                                tag="h")
                    if nch == 0:
                        nc.vector.memset(h[:], 0.0)
                    else:
                        ps = fpp.tile([P, D], F32, name=f"ps{pnum}_{t}_{rep}",
                                      tag="fps")
                        k = 0
                        for (bk, s0, n) in chunk_groups(t):
                            b = bufs[bk]
                            for si in range(s0, s0 + n):
                                nc.tensor.matmul(
                                    out=ps[:], lhsT=ident[:],
                                    rhs=b[:, si * D:(si + 1) * D],
                                    start=(k == 0), stop=(k == nch - 1))
                                k += 1
                        nc.vector.tensor_scalar(
                            out=h[:], in0=ps[:], scalar1=rdeg[:, t:t + 1],
                            scalar2=None, op0=mybir.AluOpType.mult)
                        nc.vector.tensor_tensor(
                            out=acc[:, t * D:(t + 1) * D],
                            in0=acc[:, t * D:(t + 1) * D], in1=h[:],
                            op=mybir.AluOpType.add)
                    if need_ag:
                        nc.sync.dma_start(out=hin[t * P:(t + 1) * P, :],
                                          in_=h[:])
                    if debug and pnum == 0:
                        nc.sync.dma_start(out=dbg_h0[t * P:(t + 1) * P, :],
                                          in_=h[:])

            def conv_pass(table_ap, drow, df, adst_sb, pnum, post_fn, rep):
                bufs = emit_gathers(table_ap, drow, f"c{pnum}r{rep}")
                for t in range(TPC):
                    nch = int(CLO[t] + CHI[t])
                    if nch == 0:
                        post_fn(t, None)
                        continue
                    ps = fpp.tile([P, df], F32, name=f"cp{pnum}_{t}_{rep}",
                                  tag="fps")
                    k = 0
                    for (bk, s0, n) in chunk_groups(t):
                        b = bufs[bk]
                        g3 = b[:, s0 * drow:(s0 + n) * drow].rearrange(
                            "p (c d) -> p c d", d=drow)
                        z = sp.tile([P, WIN], F32,
                                    name=f"z{pnum}_{t}_{k}_{rep}", tag="z")
                        nc.vector.tensor_scalar(
                            out=z[:, :n].rearrange("p (c u) -> p c u", u=1),
                            in0=g3[:, :, df:df + 1],
                            scalar1=adst_sb[:, t:t + 1], scalar2=None,
                            op0=mybir.AluOpType.add)
                        sc = sp.tile([P, WIN], F32,
                                     name=f"sc{pnum}_{t}_{k}_{rep}", tag="sc")
                        nc.scalar.activation(
                            out=sc[:, :n], in_=z[:, :n],
                            func=mybir.ActivationFunctionType.Lrelu,
                            alpha=NEG_SLOPE)
                        w8 = wp.tile([P, WIN * D], F32,
                                     name=f"w8{pnum}_{t}_{k}_{rep}", tag="w8")
                        nc.vector.tensor_tensor(
                            out=w8[:, : n * df].rearrange(
                                "p (c d) -> p c d", d=df),
                            in0=g3[:, :, 0:df],
                            in1=sc[:, :n].to_broadcast([P, n, df]),
                            op=mybir.AluOpType.mult)
                        for si in range(n):
                            nc.tensor.matmul(
                                out=ps[:], lhsT=ident[:],
                                rhs=w8[:, si * df:(si + 1) * df],
                                start=(k == 0), stop=(k == nch - 1))
                            k += 1
                    post_fn(t, ps)

            for rep in range(repeat):
                htab1 = dp.tile([NP, D], F32, name=f"htab1_{rep}",
                                addr_space="Shared")
                htab2 = dp.tile([NP, D], F32, name=f"htab2_{rep}",
                                addr_space="Shared")
                t1tab = dp.tile([NP, ROW1], F32, name=f"t1tab_{rep}",
                                addr_space="Shared")
                t2tab = dp.tile([NP, DH], F32, name=f"t2tab_{rep}",
                                addr_space="Shared")
                nc.sync.dma_start(out=acc[:], in_=x_shard_t[:])

                smoothing_pass(x_full.ap(), 0, rep)
                nc.gpsimd.collective_compute(
                    "AllGather", mybir.AluOpType.bypass,
                    ins=[hin.opt()], outs=[htab1.opt()], replica_groups=RG)
                smoothing_pass(htab1[:], 1, rep)
                nc.gpsimd.collective_compute(
                    "AllGather", mybir.AluOpType.bypass,
                    ins=[hin.opt()], outs=[htab2.opt()], replica_groups=RG)
                smoothing_pass(htab2[:], 2, rep, need_ag=False)

                if debug:
                    nc.sync.dma_start(out=dbg_acc[:], in_=acc[:])
                for t in range(TPC):
                    proj_tile(t, acc[:, t * D:(t + 1) * D], wcat1, D, D,
                              ROW1, t1in, adst1, params[:, 0:1],
                              f"t1_{rep}")
                nc.gpsimd.collective_compute(
                    "AllGather", mybir.AluOpType.bypass,
                    ins=[t1in.opt()], outs=[t1tab.opt()], replica_groups=RG)

                def post1(t, ps, rep=rep):
                    h1 = sp.tile([P, DH], F32, name=f"h1_{t}_{rep}", tag="h1")
                    if ps is None:
                        nc.vector.memset(h1[:], 0.0)
                    else:
                        nc.scalar.activation(
                            out=h1[:], in_=ps[:],
                            func=mybir.ActivationFunctionType.Relu)
                    if debug:
                        nc.sync.dma_start(out=dbg_h1[t * P:(t + 1) * P, :],
                                          in_=h1[:])
                    proj_tile(t, h1[:], wcat2, DH, DO, DH, t2in, adst2,
                              params[:, 1:2], f"t2_{rep}")

                conv_pass(t1tab[:], ROW1, D, adst1, 1, post1, rep)
                nc.gpsimd.collective_compute(
                    "AllGather", mybir.AluOpType.bypass,
                    ins=[t2in.opt()], outs=[t2tab.opt()], replica_groups=RG)

                def post2(t, ps, rep=rep):
                    o = sp.tile([P, DO], F32, name=f"o_{t}_{rep}", tag="o")
                    if ps is None:
                        nc.vector.memset(o[:], 0.0)
                    else:
                        nc.scalar.activation(
                            out=o[:], in_=ps[:],
                            func=mybir.ActivationFunctionType.Copy)
                    nc.sync.dma_start(out=out_sh[t * P:(t + 1) * P, :],
                                      in_=o[:])

                conv_pass(t2tab[:], DH, DO, adst2, 2, post2, rep)

    nc.compile()
    return nc


# ---------------------------------------------------------------- driver

_CACHE = {}


def _get_runner(s, D, DH, DO, repeat):
    key = (s["NP"], s["TOTC"], s["TOT16"], tuple(int(v) for v in s["CLO"]),
           tuple(int(v) for v in s["CHI"]), D, DH, DO, repeat)
    if key not in _CACHE:
        nc = _build_program(s, D, DH, DO, repeat)
        _CACHE[key] = _Runner(nc, M)
    return _CACHE[key]


def _prep_inputs(s, x, W_att1, b_att1, W_lin1, W_att2, b_att2, W_lin2):
    NP, TPC = s["NP"], s["TPC"]
    N, D = x.shape
    DH = W_lin1.shape[1]
    DO = W_lin2.shape[1]
    pi = s["pi"]

    x_full = np.zeros((NP, D), np.float32)
    x_full[pi[:N]] = x
    x_sh = x_full.reshape(M, TPC, P, D)

    wcat1 = np.concatenate(
        [W_lin1, W_att1[:D, :1], W_att1[D:, :1]], axis=1) * 0.25
    wcat2 = np.concatenate(
        [W_lin2, W_att2[:DH, :1], W_att2[DH:, :1]], axis=1)
    params = np.zeros((P, 2), np.float32)
    params[:, 0] = float(np.asarray(b_att1).reshape(-1)[0])
    params[:, 1] = float(np.asarray(b_att2).reshape(-1)[0])

    in_maps = []
    for c in range(M):
        in_maps.append({
            "x_full": x_full,
            "x_shard_t": np.ascontiguousarray(
                x_sh[c].transpose(1, 0, 2)).reshape(P, TPC * D),
            "idx_in": s["idx_res"][c],
            "rdeg_in": s["rdeg_ct"][c],
            "wcat1_in": wcat1.astype(np.float32),
            "wcat2_in": wcat2.astype(np.float32),
            "params_in": params,
        })
    return in_maps


def kernel(x, edge_index, W_att1, b_att1, W_lin1, W_att2, b_att2, W_lin2):
    x = np.asarray(x, np.float32)
    edge_index = np.asarray(edge_index)
    N, D = x.shape
    W_lin1 = np.asarray(W_lin1, np.float32)
    W_lin2 = np.asarray(W_lin2, np.float32)
    DH = W_lin1.shape[1]
    DO = W_lin2.shape[1]
    src = edge_index[0].astype(np.int64)
    dst = edge_index[1].astype(np.int64)

    s = _build_schedule(src, dst, N)
    repeat = int(os.environ.get("CC_GCN_REPEAT", "1"))
    r = _get_runner(s, D, DH, DO, repeat)
    in_maps = _prep_inputs(s, x, np.asarray(W_att1, np.float32),
                           np.asarray(b_att1, np.float32), W_lin1,
                           np.asarray(W_att2, np.float32),
                           np.asarray(b_att2, np.float32), W_lin2)
    res = r.run(in_maps)

    pi = s["pi"]
    out_pi = np.concatenate([res[c]["out_sh"] for c in range(M)], axis=0)
    return np.ascontiguousarray(out_pi[pi[:N]]).astype(np.float32)
